# revision 58
# baseline (speedup 1.0000x reference)
"""Trainium2 Bass kernel for the sparse segment-softmax attention module.

Math: per nnz k, out[k] = exp(u2[b, j_k]) / sum_{d in seg(b,i)} exp(u2[b, j_d])
with u2[b, n] = t2[b, n, :].g, g = W2^T v (the q1.v and bias terms cancel in
the segment softmax).

Gather strategy: all pool-engine gathers (IndirectCopy / ap_gather) are bound
at ~28 ns/index per 16-partition group (~115 us for the 32768 gathers each
NeuronCore owns) and dma_gather desc-gen costs ~8.4 ns/value of pool time
(all measured on HW).  Instead the gather runs on the otherwise-idle PE:
indices are kernel inputs, so the host uploads fp8 one-hot STATIONARIES.
Two 64-row one-hots are stacked per [128c x 128p] stationary (column p
selects row j%64 of block 2i or 2i+1), multiplied by a block-diagonal
moving operand [Ecol 0; 0 Ecol] ([128, 16] bf16, Ecol[c, hi] =
exp(u2[64*hi+c])) -> psum[p, 16i:16i+16] holds both blocks' 8 hi-candidates:
256 gathered values per ldweights+matmul pair (~0.2 ns/value).  A DVE
multiply with the fp8 hi-mask plus a width-8 strided reduce selects the
right candidate.  Mixed fp8-stationary x bf16-moving matmuls verified on HW.

Performance structure (measured):
  - per-dma_start sequencer issue costs ~0.6 us, so inputs ship as 11 large
    DMAs: one t2 per batch, one packed smalls (g + both masks), and the
    one-hots as 4 chunks per batch, interleaved (b0,h),(b1,h) on the sync
    ring so each 16-pair matmul phase starts as soon as its chunk lands.
  - u2 runs on the PE directly in Ecol layout (64-column stationaries from
    host-transposed fp8 t2 against g columns x256 fp8; ACT exp fuses the
    1/256 scale) -- no partition reshuffle needed.
  - gather phases alternate batches so the PE never idles (idle drops its
    p-state); each chunk's mask-select runs on the DVE under the next
    matmul phase; only a [128,128]-scale normalize (segment sum of 32,
    reciprocal, multiply) and store remain at the end.
  - effective HBM rate here is ~150-250 GB/s with a slow first ~7 us, so
    total input bytes (~3.1 MB: 1 MB t2 + 2x0.5 MB one-hots + masks) are
    kept minimal via fp8 (u2 tolerates fp8 t2/g; one-hot 1.0 is exact).

Layout: nnz (i, d) sits at C[i%128, 32*(i//128)+d]; the host applies the
fixed inverse permutation after gathering core outputs.
"""

import os
from contextlib import ExitStack

import numpy as np

B = 16
N1 = 512
N2 = 512
F2 = 1024
DEG = 32
NNZ = B * N1 * DEG
NCORES = 8
BPC = B // NCORES  # batches per core
NBLK = 128  # one-hot blocks per batch (128 nnz each)
CH = 64  # one-hot contract height (j % CH selects the row)
NHI = 512 // CH  # quadrant count for the mask select

_CACHE: dict = {}


def _build_program():
    import concourse.bacc as bacc
    import concourse.mybir as mybir
    import concourse.tile as tile

    fp32 = mybir.dt.float32
    bf16 = mybir.dt.bfloat16
    fp8 = mybir.dt.float8e4

    nc = bacc.Bacc("TRN2", target_bir_lowering=False, debug=False)

    t2t = nc.dram_tensor("t2t", [BPC, 128, 8 * N2], fp8, kind="ExternalInput")
    # sm packs gcol (cols 0:8) and both hi-masks into ONE upload
    sm = nc.dram_tensor("sm", [128, 8 + 2 * NHI * NBLK], fp8, kind="ExternalInput")
    # oh: one-hot stationaries, c-major: oh[b, c, 128t+p] = (J[p+128t] % CH == c)
    oh = nc.dram_tensor("oh", [BPC, 128, NBLK * 64], fp8, kind="ExternalInput")
    out = nc.dram_tensor("out", [BPC, 128, 128], fp32, kind="ExternalOutput")

    with tile.TileContext(nc) as tc, ExitStack() as ctx:
        constp = ctx.enter_context(tc.tile_pool(name="const", bufs=1))
        t2p = ctx.enter_context(tc.tile_pool(name="t2p", bufs=4))
        ohp = ctx.enter_context(tc.tile_pool(name="ohp", bufs=1))
        smallp = ctx.enter_context(tc.tile_pool(name="small", bufs=2))
        psum_p = ctx.enter_context(tc.tile_pool(name="psg", bufs=1, space="PSUM"))
        psum_u = ctx.enter_context(tc.tile_pool(name="psu", bufs=2, space="PSUM"))

        # gcol + both hi-masks in a single small scalar-ring DMA.
        sm_t = constp.tile([128, 8 + 2 * NHI * NBLK], fp8)
        nc.scalar.dma_start(sm_t[:], sm[:])
        g_sb = sm_t[:, 0:8]
        hm_tiles = [
            sm_t[:, 8 + b * NHI * NBLK : 8 + (b + 1) * NHI * NBLK]
            for b in range(BPC)
        ]

        # transposed-t2: ONE partition-major DMA per batch (b0 on sync, b1
        # on scalar) -- per-dma_start sequencer issue costs ~0.6 us, so the
        # kernel uses as few, as large DMAs as possible.
        # t2 pre-arranged partition-major on the host: each DMA is 128
        # contiguous 4 KB descriptors (vs 1024 strided 512 B ones), much
        # faster through the DMA ramp that gates the whole kernel head.
        def load_t2(b, eng):
            t2b = t2p.tile([128, 8 * N2], fp8, tag=f"t2_{b}", name=f"t2b{b}")
            eng.dma_start(t2b[:], t2t[b])
            return t2b

        t2_tiles = [load_t2(0, nc.sync), load_t2(1, nc.scalar)]

        # One-hot stationaries: the sync ring carries ONLY this stream,
        # as four 1 MB chunk tiles per batch so each 32-block group of
        # matmuls starts as soon as its chunk lands.
        oh_tiles = {}
        NCHUNK = 4
        OHCHUNK = NBLK * 64 // NCHUNK

        def load_chunk(b, h):
            oht = ohp.tile(
                [128, OHCHUNK], fp8, tag=f"oh{b}_{h}", name=f"oh{b}_{h}"
            )
            eng = nc.scalar if h == NCHUNK - 1 else nc.sync
            eng.dma_start(oht[:], oh[b][:, h * OHCHUNK : (h + 1) * OHCHUNK])
            oh_tiles[(b, h)] = oht

        for h in range(NCHUNK):
            for b in range(BPC):
                load_chunk(b, h)

        # u2 on the PE, directly in Ecol layout: 64-column stationaries
        # make psum[c, hi] = 256*u2[64*hi + c] (hi = n-block of 64).
        ecols = []
        for b in range(BPC):
            upsum = psum_u.tile([CH, NHI], fp32, tag="upsum")
            for col in range(NHI):
                for q in range(8):
                    base = 512 * q + CH * col
                    nc.tensor.matmul(
                        upsum[:, col : col + 1],
                        t2_tiles[b][:, base : base + CH],
                        g_sb[:, q : q + 1],
                        start=(q == 0),
                        stop=(q == 7),
                    )
            # block-diagonal moving operand: [Ecol 0; 0 Ecol] so one
            # [128, 128] stationary gathers TWO stacked 64-row one-hots
            ecol = smallp.tile([128, 2 * NHI], bf16, tag=f"ecol{b}", name=f"ecol{b}")
            nc.vector.memset(ecol[:], 0.0)
            nc.scalar.activation(
                ecol[0:CH, 0:NHI],
                upsum[:],
                func=mybir.ActivationFunctionType.Exp,
                scale=1.0 / 256.0,
            )
            nc.scalar.activation(
                ecol[CH:128, NHI : 2 * NHI],
                upsum[:],
                func=mybir.ActivationFunctionType.Exp,
                scale=1.0 / 256.0,
            )
            ecols.append(ecol)

        # Gather matmuls in four chunk-phases interleaved across batches so
        # the PE never idles (idling drops its p-state) and batch 0's DVE
        # select overlaps batch 1's last phase.
        psums = [
            psum_p.tile([128, NHI * NBLK], fp32, tag=f"ps{b}", name=f"psg{b}")
            for b in range(BPC)
        ]

        PPC = NBLK // 2 // NCHUNK  # block-pairs per chunk
        def gather_phase(b, h, lo=0, hi=None):
            hi = PPC if hi is None else hi
            for tt in range(lo, hi):
                i = PPC * h + tt
                nc.tensor.matmul(
                    psums[b][:, 2 * NHI * i : 2 * NHI * (i + 1)],
                    oh_tiles[(b, h)][:, 128 * tt : 128 * (tt + 1)],
                    ecols[b][:],
                    start=True,
                    stop=True,
                )

        cs = []
        for b in range(BPC):
            cb = smallp.tile([128, NBLK], fp32, tag=f"C{b}", name=f"C{b}")
            cs.append(cb)

        def select_part(b, p0, p1):
            # mask-select pairs [p0, p1) of batch b right after their
            # matmuls so the DVE work hides under the next matmul phase
            w = 2 * NHI * (p1 - p0)
            sel = smallp.tile([128, 2 * NHI * PPC], fp32, tag="selh", name="selh")
            nc.vector.tensor_tensor(
                out=sel[:, 0:w],
                in0=psums[b][:, 2 * NHI * p0 : 2 * NHI * p1],
                in1=hm_tiles[b][:, 2 * NHI * p0 : 2 * NHI * p1],
                op=mybir.AluOpType.mult,
            )
            nc.vector.tensor_reduce(
                out=cs[b][:, 2 * p0 : 2 * p1],
                in_=sel[:, 0:w].rearrange("p (t x) -> p t x", x=NHI),
                axis=mybir.AxisListType.X,
                op=mybir.AluOpType.add,
            )

        os_ = {}

        def finalize_windows(b, q0, q1, store=False):
            # normalize segment windows [q0, q1) of batch b; windows are
            # chunk-aligned, so windows 0..2 finalize before the last
            # gather phase and only window 3 remains in the tail
            c3 = cs[b][:].rearrange("p (q d) -> p q d", d=DEG)[:, q0:q1, :]
            sden = smallp.tile([128, 4], fp32, tag=f"S{b}", name=f"S{b}")
            nc.vector.tensor_reduce(
                out=sden[:, q0:q1], in_=c3, axis=mybir.AxisListType.X,
                op=mybir.AluOpType.add,
            )
            r = smallp.tile([128, 4], fp32, tag=f"R{b}", name=f"R{b}")
            nc.vector.reciprocal(r[:, q0:q1], sden[:, q0:q1])
            if b not in os_:
                os_[b] = smallp.tile([128, NBLK], fp32, tag=f"O{b}", name=f"O{b}")
            o = os_[b]
            o3 = o[:].rearrange("p (q d) -> p q d", d=DEG)[:, q0:q1, :]
            r3 = r[:, q0:q1].unsqueeze(2).broadcast_to((128, q1 - q0, DEG))
            nc.vector.tensor_tensor(out=o3, in0=c3, in1=r3, op=mybir.AluOpType.mult)
            if store:
                nc.scalar.dma_start(out[b], o[:])

        for h in range(NCHUNK):
            for b in range(BPC):
                if (b, h) == (1, NCHUNK - 1):
                    finalize_windows(0, 0, 4, store=True)
                    finalize_windows(1, 0, 3)
                    # split the final phase so the tail select after the
                    # very last matmul covers only 4 pairs
                    gather_phase(b, h, 0, PPC - 4)
                    select_part(b, PPC * h, PPC * (h + 1) - 4)
                    gather_phase(b, h, PPC - 4, PPC)
                    select_part(b, PPC * (h + 1) - 4, PPC * (h + 1))
                else:
                    gather_phase(b, h)
                    select_part(b, PPC * h, PPC * (h + 1))
        finalize_windows(1, 3, 4, store=True)

    nc.compile()
    return nc


def _prep_core_inputs(t2, idx_j, W2, v):
    import ml_dtypes

    bf16 = ml_dtypes.bfloat16
    fp8 = ml_dtypes.float8_e4m3fn
    g = (W2.T.astype(np.float64) @ v.astype(np.float64)).astype(np.float32)
    gcol = (g * 256.0).reshape(8, 128).T.astype(fp8)
    t2t = t2.transpose(0, 2, 1).astype(fp8)  # [B, F2, N2]
    t2t = np.ascontiguousarray(
        t2t.reshape(B, 8, 128, N2).transpose(0, 2, 1, 3).reshape(B, 128, 8 * N2)
    )

    # nnz (i, d) lands at C[p, t]: p = i % 128, t = 32*(i//128) + d
    i_arr = np.arange(N1)
    d_arr = np.arange(DEG)
    tt = (DEG * (i_arr[:, None] // 128) + d_arr[None, :])  # [512, 32]
    pp = np.broadcast_to((i_arr[:, None] % 128), (N1, DEG))

    j3 = np.asarray(idx_j).reshape(B, N1, DEG)
    in_maps = []
    eye = np.eye(CH, dtype=fp8)
    hvals = np.arange(NHI, dtype=np.int32)
    for c in range(NCORES):
        bb = slice(BPC * c, BPC * (c + 1))
        ohs = np.empty((BPC, 128, NBLK * 64), dtype=fp8)
        hms = np.empty((BPC, 128, NHI * NBLK), dtype=fp8)
        for lb in range(BPC):
            gb = BPC * c + lb
            jmat = np.empty((128, NBLK), dtype=np.int32)  # jmat[p, t] = J
            jmat[pp.ravel(), tt.ravel()] = j3[gb].ravel()
            lo = jmat % CH
            hi = jmat // CH
            # stack block pairs: rows 0:64 = block 2i, 64:128 = block 2i+1
            o4 = eye[:, lo.T].reshape(CH, NBLK // 2, 2, 128)
            ohs[lb] = o4.transpose(2, 0, 1, 3).reshape(128, NBLK * 64)
            hms[lb] = (hi[:, :, None] == hvals).astype(fp8).reshape(128, NHI * NBLK)
        smv = np.concatenate([gcol, hms[0], hms[1]], axis=1)
        in_maps.append(
            {
                "t2t": np.ascontiguousarray(t2t[bb]),
                "sm": np.ascontiguousarray(smv),
                "oh": ohs,
            }
        )
    return in_maps


def kernel(t1, t2, idx_b, idx_i, idx_j, W1, b1, W2, b2, v):
    from concourse.bass_utils import run_bass_kernel_spmd

    if "nc" not in _CACHE:
        _CACHE["nc"] = _build_program()
    nc = _CACHE["nc"]

    in_maps = _prep_core_inputs(
        np.asarray(t2, dtype=np.float32),
        np.asarray(idx_j),
        np.asarray(W2, dtype=np.float32),
        np.asarray(v, dtype=np.float32),
    )
    trace = bool(int(os.environ.get("KERNEL_TRACE", "0")))
    last_err = None
    for _attempt in range(3):
        try:
            res = run_bass_kernel_spmd(nc, in_maps, list(range(NCORES)), trace=trace)
            break
        except Exception as e:  # transient NRT_EXEC_UNIT_UNRECOVERABLE wedges
            last_err = e
    else:
        raise last_err
    _CACHE["last_results"] = res
    outs = []
    for r in res.results:
        o = r["out"].reshape(BPC, 128, 4, DEG)  # [b, p, q, d]
        o = o.transpose(0, 2, 1, 3).reshape(BPC * N1 * DEG)  # i = 128q + p
        outs.append(o)
    return np.concatenate(outs).astype(np.float32)


# revision 59
# speedup vs baseline: 1.0250x; 1.0250x over previous
"""Trainium2 Bass kernel for the sparse segment-softmax attention module.

Math: per nnz k, out[k] = exp(u2[b, j_k]) / sum_{d in seg(b,i)} exp(u2[b, j_d])
with u2[b, n] = t2[b, n, :].g, g = W2^T v (the q1.v and bias terms cancel in
the segment softmax).

Gather strategy: all pool-engine gathers (IndirectCopy / ap_gather) are bound
at ~28 ns/index per 16-partition group (~115 us for the 32768 gathers each
NeuronCore owns) and dma_gather desc-gen costs ~8.4 ns/value of pool time
(all measured on HW).  Instead the gather runs on the otherwise-idle PE:
indices are kernel inputs, so the host uploads fp8 one-hot STATIONARIES.
Two 64-row one-hots are stacked per [128c x 128p] stationary (column p
selects row j%64 of block 2i or 2i+1), multiplied by a block-diagonal
moving operand [Ecol 0; 0 Ecol] ([128, 16] bf16, Ecol[c, hi] =
exp(u2[64*hi+c])) -> psum[p, 16i:16i+16] holds both blocks' 8 hi-candidates:
256 gathered values per ldweights+matmul pair (~0.2 ns/value).  A DVE
multiply with the fp8 hi-mask plus a width-8 strided reduce selects the
right candidate.  Mixed fp8-stationary x bf16-moving matmuls verified on HW.

Performance structure (measured):
  - per-dma_start sequencer issue costs ~0.6 us, so inputs ship as 11 large
    DMAs: one t2 per batch, one packed smalls (g + both masks), and the
    one-hots as 4 chunks per batch, interleaved (b0,h),(b1,h) on the sync
    ring so each 16-pair matmul phase starts as soon as its chunk lands.
  - u2 runs on the PE directly in Ecol layout (64-column stationaries from
    host-transposed fp8 t2 against g columns x256 fp8; ACT exp fuses the
    1/256 scale) -- no partition reshuffle needed.
  - gather phases alternate batches so the PE never idles (idle drops its
    p-state); each chunk's mask-select runs on the DVE under the next
    matmul phase; only a [128,128]-scale normalize (segment sum of 32,
    reciprocal, multiply) and store remain at the end.
  - effective HBM rate here is ~150-250 GB/s with a slow first ~7 us, so
    total input bytes (~3.1 MB: 1 MB t2 + 2x0.5 MB one-hots + masks) are
    kept minimal via fp8 (u2 tolerates fp8 t2/g; one-hot 1.0 is exact).

Layout: nnz (i, d) sits at C[i%128, 32*(i//128)+d]; the host applies the
fixed inverse permutation after gathering core outputs.
"""

import os
from contextlib import ExitStack

import numpy as np

B = 16
N1 = 512
N2 = 512
F2 = 1024
DEG = 32
NNZ = B * N1 * DEG
NCORES = 8
BPC = B // NCORES  # batches per core
NBLK = 128  # one-hot blocks per batch (128 nnz each)
CH = 64  # one-hot contract height (j % CH selects the row)
NHI = 512 // CH  # quadrant count for the mask select

_CACHE: dict = {}


def _build_program():
    import concourse.bacc as bacc
    import concourse.mybir as mybir
    import concourse.tile as tile

    fp32 = mybir.dt.float32
    bf16 = mybir.dt.bfloat16
    fp8 = mybir.dt.float8e4

    nc = bacc.Bacc("TRN2", target_bir_lowering=False, debug=False)

    t2t = nc.dram_tensor("t2t", [BPC, 128, 8 * N2], fp8, kind="ExternalInput")
    # sm packs gcol (cols 0:8) and both hi-masks into ONE upload
    sm = nc.dram_tensor("sm", [128, 8 + 2 * NHI * NBLK], fp8, kind="ExternalInput")
    # oh: one-hot stationaries, c-major: oh[b, c, 128t+p] = (J[p+128t] % CH == c)
    oh = nc.dram_tensor("oh", [BPC, 128, NBLK * 64], fp8, kind="ExternalInput")
    out = nc.dram_tensor("out", [BPC, 128, 128], fp32, kind="ExternalOutput")

    with tile.TileContext(nc) as tc, ExitStack() as ctx:
        constp = ctx.enter_context(tc.tile_pool(name="const", bufs=1))
        t2p = ctx.enter_context(tc.tile_pool(name="t2p", bufs=4))
        ohp = ctx.enter_context(tc.tile_pool(name="ohp", bufs=1))
        smallp = ctx.enter_context(tc.tile_pool(name="small", bufs=2))
        psum_p = ctx.enter_context(tc.tile_pool(name="psg", bufs=1, space="PSUM"))
        psum_u = ctx.enter_context(tc.tile_pool(name="psu", bufs=2, space="PSUM"))

        # gcol + both hi-masks in a single small scalar-ring DMA.
        sm_t = constp.tile([128, 8 + 2 * NHI * NBLK], fp8)
        nc.scalar.dma_start(sm_t[:], sm[:])
        g_sb = sm_t[:, 0:8]
        hm_tiles = [
            sm_t[:, 8 + b * NHI * NBLK : 8 + (b + 1) * NHI * NBLK]
            for b in range(BPC)
        ]

        # transposed-t2: ONE partition-major DMA per batch (b0 on sync, b1
        # on scalar) -- per-dma_start sequencer issue costs ~0.6 us, so the
        # kernel uses as few, as large DMAs as possible.
        # t2 pre-arranged partition-major on the host: each DMA is 128
        # contiguous 4 KB descriptors (vs 1024 strided 512 B ones), much
        # faster through the DMA ramp that gates the whole kernel head.
        def load_t2(b, eng):
            t2b = t2p.tile([128, 8 * N2], fp8, tag=f"t2_{b}", name=f"t2b{b}")
            eng.dma_start(t2b[:], t2t[b])
            return t2b

        t2_tiles = [load_t2(0, nc.sync), load_t2(1, nc.sync)]

        # One-hot stationaries: the sync ring carries ONLY this stream,
        # as four 1 MB chunk tiles per batch so each 32-block group of
        # matmuls starts as soon as its chunk lands.
        oh_tiles = {}
        NCHUNK = 4
        OHCHUNK = NBLK * 64 // NCHUNK

        def load_chunk(b, h):
            oht = ohp.tile(
                [128, OHCHUNK], fp8, tag=f"oh{b}_{h}", name=f"oh{b}_{h}"
            )
            # chunk (0,0) rides the scalar ring right behind sm so the
            # first gather phase starts ~1 us earlier; last chunks too
            eng = (
                nc.scalar
                if (h == NCHUNK - 1 or (b, h) == (0, 0))
                else nc.sync
            )
            eng.dma_start(oht[:], oh[b][:, h * OHCHUNK : (h + 1) * OHCHUNK])
            oh_tiles[(b, h)] = oht

        for h in range(NCHUNK):
            for b in range(BPC):
                load_chunk(b, h)

        # u2 on the PE, directly in Ecol layout: 64-column stationaries
        # make psum[c, hi] = 256*u2[64*hi + c] (hi = n-block of 64).
        ecols = []
        for b in range(BPC):
            upsum = psum_u.tile([CH, NHI], fp32, tag="upsum")
            for col in range(NHI):
                for q in range(8):
                    base = 512 * q + CH * col
                    nc.tensor.matmul(
                        upsum[:, col : col + 1],
                        t2_tiles[b][:, base : base + CH],
                        g_sb[:, q : q + 1],
                        start=(q == 0),
                        stop=(q == 7),
                    )
            # block-diagonal moving operand: [Ecol 0; 0 Ecol] so one
            # [128, 128] stationary gathers TWO stacked 64-row one-hots
            ecol = smallp.tile([128, 2 * NHI], bf16, tag=f"ecol{b}", name=f"ecol{b}")
            nc.vector.memset(ecol[:], 0.0)
            nc.scalar.activation(
                ecol[0:CH, 0:NHI],
                upsum[:],
                func=mybir.ActivationFunctionType.Exp,
                scale=1.0 / 256.0,
            )
            nc.scalar.activation(
                ecol[CH:128, NHI : 2 * NHI],
                upsum[:],
                func=mybir.ActivationFunctionType.Exp,
                scale=1.0 / 256.0,
            )
            ecols.append(ecol)

        # Gather matmuls in four chunk-phases interleaved across batches so
        # the PE never idles (idling drops its p-state) and batch 0's DVE
        # select overlaps batch 1's last phase.
        psums = [
            psum_p.tile([128, NHI * NBLK], fp32, tag=f"ps{b}", name=f"psg{b}")
            for b in range(BPC)
        ]

        PPC = NBLK // 2 // NCHUNK  # block-pairs per chunk
        def gather_phase(b, h, lo=0, hi=None):
            hi = PPC if hi is None else hi
            for tt in range(lo, hi):
                i = PPC * h + tt
                nc.tensor.matmul(
                    psums[b][:, 2 * NHI * i : 2 * NHI * (i + 1)],
                    oh_tiles[(b, h)][:, 128 * tt : 128 * (tt + 1)],
                    ecols[b][:],
                    start=True,
                    stop=True,
                )

        cs = []
        for b in range(BPC):
            cb = smallp.tile([128, NBLK], fp32, tag=f"C{b}", name=f"C{b}")
            cs.append(cb)

        def select_part(b, p0, p1):
            # mask-select pairs [p0, p1) of batch b right after their
            # matmuls so the DVE work hides under the next matmul phase
            w = 2 * NHI * (p1 - p0)
            sel = smallp.tile([128, 2 * NHI * PPC], fp32, tag="selh", name="selh")
            nc.vector.tensor_tensor(
                out=sel[:, 0:w],
                in0=psums[b][:, 2 * NHI * p0 : 2 * NHI * p1],
                in1=hm_tiles[b][:, 2 * NHI * p0 : 2 * NHI * p1],
                op=mybir.AluOpType.mult,
            )
            nc.vector.tensor_reduce(
                out=cs[b][:, 2 * p0 : 2 * p1],
                in_=sel[:, 0:w].rearrange("p (t x) -> p t x", x=NHI),
                axis=mybir.AxisListType.X,
                op=mybir.AluOpType.add,
            )

        os_ = {}

        def finalize_windows(b, q0, q1, store=False):
            # normalize segment windows [q0, q1) of batch b; windows are
            # chunk-aligned, so windows 0..2 finalize before the last
            # gather phase and only window 3 remains in the tail
            c3 = cs[b][:].rearrange("p (q d) -> p q d", d=DEG)[:, q0:q1, :]
            sden = smallp.tile([128, 4], fp32, tag=f"S{b}", name=f"S{b}")
            nc.vector.tensor_reduce(
                out=sden[:, q0:q1], in_=c3, axis=mybir.AxisListType.X,
                op=mybir.AluOpType.add,
            )
            r = smallp.tile([128, 4], fp32, tag=f"R{b}", name=f"R{b}")
            nc.vector.reciprocal(r[:, q0:q1], sden[:, q0:q1])
            if b not in os_:
                os_[b] = smallp.tile([128, NBLK], fp32, tag=f"O{b}", name=f"O{b}")
            o = os_[b]
            o3 = o[:].rearrange("p (q d) -> p q d", d=DEG)[:, q0:q1, :]
            r3 = r[:, q0:q1].unsqueeze(2).broadcast_to((128, q1 - q0, DEG))
            nc.vector.tensor_tensor(out=o3, in0=c3, in1=r3, op=mybir.AluOpType.mult)
            if store:
                nc.scalar.dma_start(out[b], o[:])

        for h in range(NCHUNK):
            for b in range(BPC):
                if (b, h) == (1, NCHUNK - 1):
                    finalize_windows(0, 0, 4, store=True)
                    finalize_windows(1, 0, 3)
                    # split the final phase so the tail select after the
                    # very last matmul covers only 4 pairs
                    gather_phase(b, h, 0, PPC - 4)
                    select_part(b, PPC * h, PPC * (h + 1) - 4)
                    gather_phase(b, h, PPC - 4, PPC)
                    select_part(b, PPC * (h + 1) - 4, PPC * (h + 1))
                else:
                    gather_phase(b, h)
                    select_part(b, PPC * h, PPC * (h + 1))
        finalize_windows(1, 3, 4, store=True)

    nc.compile()
    return nc


def _prep_core_inputs(t2, idx_j, W2, v):
    import ml_dtypes

    bf16 = ml_dtypes.bfloat16
    fp8 = ml_dtypes.float8_e4m3fn
    g = (W2.T.astype(np.float64) @ v.astype(np.float64)).astype(np.float32)
    gcol = (g * 256.0).reshape(8, 128).T.astype(fp8)
    t2t = t2.transpose(0, 2, 1).astype(fp8)  # [B, F2, N2]
    t2t = np.ascontiguousarray(
        t2t.reshape(B, 8, 128, N2).transpose(0, 2, 1, 3).reshape(B, 128, 8 * N2)
    )

    # nnz (i, d) lands at C[p, t]: p = i % 128, t = 32*(i//128) + d
    i_arr = np.arange(N1)
    d_arr = np.arange(DEG)
    tt = (DEG * (i_arr[:, None] // 128) + d_arr[None, :])  # [512, 32]
    pp = np.broadcast_to((i_arr[:, None] % 128), (N1, DEG))

    j3 = np.asarray(idx_j).reshape(B, N1, DEG)
    in_maps = []
    eye = np.eye(CH, dtype=fp8)
    hvals = np.arange(NHI, dtype=np.int32)
    for c in range(NCORES):
        bb = slice(BPC * c, BPC * (c + 1))
        ohs = np.empty((BPC, 128, NBLK * 64), dtype=fp8)
        hms = np.empty((BPC, 128, NHI * NBLK), dtype=fp8)
        for lb in range(BPC):
            gb = BPC * c + lb
            jmat = np.empty((128, NBLK), dtype=np.int32)  # jmat[p, t] = J
            jmat[pp.ravel(), tt.ravel()] = j3[gb].ravel()
            lo = jmat % CH
            hi = jmat // CH
            # stack block pairs: rows 0:64 = block 2i, 64:128 = block 2i+1
            o4 = eye[:, lo.T].reshape(CH, NBLK // 2, 2, 128)
            ohs[lb] = o4.transpose(2, 0, 1, 3).reshape(128, NBLK * 64)
            hms[lb] = (hi[:, :, None] == hvals).astype(fp8).reshape(128, NHI * NBLK)
        smv = np.concatenate([gcol, hms[0], hms[1]], axis=1)
        in_maps.append(
            {
                "t2t": np.ascontiguousarray(t2t[bb]),
                "sm": np.ascontiguousarray(smv),
                "oh": ohs,
            }
        )
    return in_maps


def kernel(t1, t2, idx_b, idx_i, idx_j, W1, b1, W2, b2, v):
    from concourse.bass_utils import run_bass_kernel_spmd

    if "nc" not in _CACHE:
        _CACHE["nc"] = _build_program()
    nc = _CACHE["nc"]

    in_maps = _prep_core_inputs(
        np.asarray(t2, dtype=np.float32),
        np.asarray(idx_j),
        np.asarray(W2, dtype=np.float32),
        np.asarray(v, dtype=np.float32),
    )
    trace = bool(int(os.environ.get("KERNEL_TRACE", "0")))
    last_err = None
    for _attempt in range(3):
        try:
            res = run_bass_kernel_spmd(nc, in_maps, list(range(NCORES)), trace=trace)
            break
        except Exception as e:  # transient NRT_EXEC_UNIT_UNRECOVERABLE wedges
            last_err = e
    else:
        raise last_err
    _CACHE["last_results"] = res
    outs = []
    for r in res.results:
        o = r["out"].reshape(BPC, 128, 4, DEG)  # [b, p, q, d]
        o = o.transpose(0, 2, 1, 3).reshape(BPC * N1 * DEG)  # i = 128q + p
        outs.append(o)
    return np.concatenate(outs).astype(np.float32)
